# revision 1
# baseline (speedup 1.0000x reference)
"""Multi-head causal self-attention on 8 Trainium2 NeuronCores.

Problem: B=8, T=1024, D=1024, H=16 heads, DH=64.
    q,k,v = einsum('btd,hdk->bhtk', x, W{q,k,v})
    scores = q @ k.T / sqrt(DH), causal mask, softmax
    out = (softmax @ v) reshaped -> [B,T,H*DH] @ Wo + bo

Sharding: batch-parallel, one batch element per core (B == n_cores == 8).
No collectives needed; weights are replicated to every core.

Per-core dataflow (transpose-free):
  xT [d,t] (host-transposed) and W matrices live with d on partitions, so
  QT/KT come out as [dh, t] (heads pair-packed to fill 128 partitions) and
  V as [t, dh] (heads quad-packed for free-dim >=256, which fp32r needs to
  run at 1 cycle/row).  Scores are computed transposed, ST[s,q] = KT.T@QT,
  so no P transpose is needed for the AV matmul: OT[dh,q] = (V|1).T @ exp(ST).
  A ones column appended to V yields the softmax denominator for free in
  row 64 of the AV psum.  exp() is applied without max-subtraction (scores
  are O(5) for randn inputs; exp stays well inside fp32 range) and causal
  masking zeroes exp(S) after the fact, so no -inf handling is needed.
  Normalization divides OT by the broadcast denominator, and the Wo
  projection consumes OT directly as the stationary operand, producing
  final[q,d] which DMAs out contiguously.

All matmuls run in float32r (TF32-like, ~1.2e-4 rel err, 4x faster than
fp32).  This walrus build only allows ONE sync-wait per instruction, so a
post-scheduling pass hoists extra waits onto inserted PE no-ops.
"""

import sys

for _p in ("/opt/trn_rl_repo", "/root/.axon_site/_ro/trn_rl_repo"):
    if _p not in sys.path:
        sys.path.insert(0, _p)

import numpy as np

import concourse.bass as bass
import concourse.mybir as mybir
import concourse.tile as tile

f32 = mybir.dt.float32
f32r = mybir.dt.float32r

B, T, D, H, DH = 8, 1024, 1024, 16, 64
NP = 128            # partitions
NC = 512            # matmul free-dim chunk (fp32 moving-operand max)
KT_ = D // NP       # 8 contraction tiles over d
NT = T // NP        # 8 tiles over t (s and q tiles)
NCH = T // NC       # 2 free-dim chunks over q
NPAIR = H // 2      # 8 head pairs   (QT/KT pack 2 heads on partitions)
NQUAD = H // 4      # 4 head quads   (V packs 4 heads on free dim)


def build_nc(split_waits=True):
    nc = bass.Bass(trn_type="TRN2")
    xt = nc.dram_tensor("xt", [D, T], f32r, kind="ExternalInput")
    wq = nc.dram_tensor("wq", [D, H * DH], f32r, kind="ExternalInput")
    wk = nc.dram_tensor("wk", [D, H * DH], f32r, kind="ExternalInput")
    wv = nc.dram_tensor("wv", [D, H * DH], f32r, kind="ExternalInput")
    wo = nc.dram_tensor("wo", [H * DH, D], f32r, kind="ExternalInput")
    bo = nc.dram_tensor("bo", [1, D], f32, kind="ExternalInput")
    out = nc.dram_tensor("out", [T, D], f32, kind="ExternalOutput")

    with tile.TileContext(nc) as tc:
        _mha(tc, nc, xt, wq, wk, wv, wo, bo, out)

    if split_waits:
        _split_waits(nc)
    return nc


def _mha(tc, nc, xt, wq, wk, wv, wo, bo, out):
    import contextlib

    ctx = contextlib.ExitStack()
    singles = ctx.enter_context(tc.tile_pool(name="singles", bufs=1))
    bigpool = ctx.enter_context(tc.tile_pool(name="bigpool", bufs=1))
    wpool = ctx.enter_context(tc.tile_pool(name="wpool", bufs=1))
    qkpool = ctx.enter_context(tc.tile_pool(name="qkpool", bufs=2))
    vpool = ctx.enter_context(tc.tile_pool(name="vpool", bufs=2))
    pexpool = ctx.enter_context(tc.tile_pool(name="pexpool", bufs=7))
    avpool = ctx.enter_context(tc.tile_pool(name="avpool", bufs=8))
    smalls = ctx.enter_context(tc.tile_pool(name="smalls", bufs=9))
    bcpool = ctx.enter_context(tc.tile_pool(name="bcpool", bufs=2))
    fpool = ctx.enter_context(tc.tile_pool(name="fpool", bufs=2))
    ps_proj = ctx.enter_context(tc.tile_pool(name="ps_proj", bufs=2, space="PSUM"))
    ps_st = ctx.enter_context(tc.tile_pool(name="ps_st", bufs=4, space="PSUM"))
    ps_av = ctx.enter_context(tc.tile_pool(name="ps_av", bufs=2, space="PSUM"))

    def act_recip(out_ap, in_ap):
        """ACT-engine reciprocal via raw InstActivation (nc.scalar.activation
        refuses Reciprocal; measured 1.5e-6 rel err on our denominator range,
        and 4.6x cheaper than the single-lane DVE reciprocal)."""
        ins = [nc.scalar.lower_ap(in_ap)]
        for arg in (0.0, 1.0, 0.0):                     # bias, scale, alpha
            ins.append(mybir.ImmediateValue(dtype=f32, value=arg))
        nc.scalar.add_instruction(mybir.InstActivation(
            name=nc.get_next_instruction_name(),
            func=mybir.ActivationFunctionType.Reciprocal,
            ins=ins,
            outs=[nc.scalar.lower_ap(out_ap)],
        ))

    with ctx:
        # --- resident inputs -------------------------------------------------
        # (memset can't write f32r directly; memset f32 then round via copy)
        onesf = singles.tile([NP, 1], f32)
        nc.vector.memset(onesf, 1.0)
        ones_row = singles.tile([1, DH], f32r)           # K=1 bcast matmul lhsT
        nc.vector.tensor_copy(out=ones_row, in_=onesf[0:1, 0:1].to_broadcast((1, DH)))
        # x^T and Wo share one 4MB slot: Wo is only needed after the last
        # QKV projection has consumed x^T
        xt_sb = bigpool.tile([NP, KT_, T], f32r, tag="big", name="xt_sb")
        nc.sync.dma_start(out=xt_sb, in_=xt.rearrange("(kt p) t -> p kt t", p=NP))
        bo_bc = singles.tile([NP, D], f32)               # bias broadcast to rows
        nc.sync.dma_start(out=bo_bc, in_=bo[0:1, :].to_broadcast((NP, D)))

        # out^T accumulator for all heads: [dh(pair-packed), pair, q]
        ot_sb = singles.tile([NP, NPAIR, T], f32r)

        # deferred normalization: (avsb, den_sb, pair, hh, c) per head-chunk;
        # flushed inside the NEXT quad's projection phase (PE never waits on
        # the reciprocal chain, and ACT batches recips = 2 table switches)
        norm_pending = []

        def flush_normalizes():
            items = list(norm_pending)
            norm_pending.clear()
            recips = []
            for avsb, den_sb, pair, hh, c in items:
                recip_sb = smalls.tile([1, NC], f32r, tag="recip", name="recip_sb")
                act_recip(recip_sb, den_sb)
                recips.append(recip_sb)
            for (avsb, den_sb, pair, hh, c), recip_sb in zip(items, recips):
                bc_ps = ps_st.tile([DH, NC], f32, tag="st_ps", name="bc_ps")
                nc.tensor.matmul(
                    out=bc_ps, lhsT=ones_row, rhs=recip_sb,
                    start=True, stop=True)
                bcast = bcpool.tile([DH, NC], f32, tag="bcast", name="bcast")
                nc.vector.tensor_copy(out=bcast, in_=bc_ps)
                nc.vector.tensor_mul(
                    out=ot_sb[hh * DH:(hh + 1) * DH, pair, c * NC:(c + 1) * NC],
                    in0=avsb,
                    in1=bcast,
                )

        wo_sb_holder = []

        for quad in range(NQUAD):
            cs = quad * 4 * DH                          # column start in w mats
            wq_sb = wpool.tile([NP, KT_, 4 * DH], f32r, tag="wq")
            wk_sb = wpool.tile([NP, KT_, 4 * DH], f32r, tag="wk")
            wv_sb = wpool.tile([NP, KT_, 4 * DH], f32r, tag="wv")
            nc.sync.dma_start(
                out=wq_sb, in_=wq[:, cs:cs + 4 * DH].rearrange("(kt p) c -> p kt c", p=NP))
            nc.sync.dma_start(
                out=wk_sb, in_=wk[:, cs:cs + 4 * DH].rearrange("(kt p) c -> p kt c", p=NP))
            nc.sync.dma_start(
                out=wv_sb, in_=wv[:, cs:cs + 4 * DH].rearrange("(kt p) c -> p kt c", p=NP))

            # --- QT / KT projections: [2*DH(partitions), T] per head pair ----
            qk_tiles = {}
            for name, w_sb in (("q", wq_sb), ("k", wk_sb)):
                for pp in range(2):                      # pair within quad
                    t_sb = qkpool.tile([NP, T], f32r, tag=f"{name}t", name=f"{name}t_sb")
                    for c in range(NCH):
                        psum = ps_proj.tile([NP, NC], f32, name="proj_ps")
                        for kd in range(KT_):
                            nc.tensor.matmul(
                                out=psum,
                                lhsT=w_sb[:, kd, pp * NP:(pp + 1) * NP],
                                rhs=xt_sb[:, kd, c * NC:(c + 1) * NC],
                                start=(kd == 0), stop=(kd == KT_ - 1),
                            )
                        nc.vector.tensor_copy(out=t_sb[:, c * NC:(c + 1) * NC], in_=psum)
                    qk_tiles[(name, pp)] = t_sb

            # previous quad's softmax normalizations run here, hidden under
            # the projection matmul stream
            if norm_pending:
                flush_normalizes()

            # --- V (+ones col): [t(partitions), head, s-tile, DH+1] ----------
            v1_sb = vpool.tile([NP, 4, NT, DH + 1], f32r)
            nc.vector.tensor_copy(
                out=v1_sb[:, :, :, DH:DH + 1],
                in_=onesf.to_broadcast((NP, 4, NT, 1)))
            for tt in range(NT):
                psum = ps_proj.tile([NP, 4 * DH], f32, name="vproj_ps", tag="proj_ps")
                for kd in range(KT_):
                    nc.tensor.matmul(
                        out=psum,
                        lhsT=xt_sb[:, kd, tt * NP:(tt + 1) * NP],
                        rhs=wv_sb[:, kd, :],
                        start=(kd == 0), stop=(kd == KT_ - 1),
                    )
                for h in range(4):
                    nc.vector.tensor_copy(
                        out=v1_sb[:, h, tt, 0:DH], in_=psum[:, h * DH:(h + 1) * DH])

            if quad == NQUAD - 1:
                # Wo reuses x^T's slot (x^T fully consumed by the V matmuls
                # above); the 4MB DMA overlaps this quad's attention phase
                wo_sb = bigpool.tile([NP, KT_, D], f32r, tag="big", name="wo_sb")
                nc.sync.dma_start(
                    out=wo_sb, in_=wo.rearrange("(kt p) d -> p kt d", p=NP))
                wo_sb_holder.append(wo_sb)

            # --- attention: scores+exp+AV pipelined at the s-tile level ------
            # diagonal blocks only compute their live columns (causal trim);
            # AV matmuls for s-tile j-1 are emitted after the score matmuls
            # for s-tile j so PE overlaps ACT's exp / GPSIMD's mask-select
            for pp in range(2):
                pair = quad * 2 + pp
                qt = qk_tiles[("q", pp)]
                kt = qk_tiles[("k", pp)]
                for c in range(NCH):
                    jmax = 4 * c + 4                    # causal: s-tiles 0..jmax-1
                    av = [ps_av.tile([NP, NC], f32, name="av_ps", tag="av_ps")
                          for _ in range(2)]

                    def _emit_st(j):
                        co = min(max(0, j - 4 * c) * NP, NC - 256)  # col trim
                        st_ps = []
                        for hh in range(2):             # head within pair
                            hp = hh * DH                # partition offset (0|64)
                            st_psum = ps_st.tile([NP, NC], f32, name="st_ps")
                            nc.tensor.matmul(
                                out=st_psum[:, co:NC],
                                lhsT=kt[hp:hp + DH, j * NP:(j + 1) * NP],
                                rhs=qt[hp:hp + DH, c * NC + co:(c + 1) * NC],
                                start=True, stop=True,
                            )
                            st_ps.append(st_psum)
                        outp = []
                        for hh in range(2):
                            p_sb = pexpool.tile([NP, NC], f32r, name="p_sb")
                            nc.scalar.activation(
                                out=p_sb[:, co:NC], in_=st_ps[hh][:, co:NC],
                                func=mybir.ActivationFunctionType.Exp)
                            if j >= 4 * c:              # diagonal block: mask
                                nc.gpsimd.affine_select(
                                    out=p_sb[:, co:NC], in_=p_sb[:, co:NC],
                                    pattern=[[1, NC - co]],
                                    compare_op=mybir.AluOpType.is_ge,
                                    fill=0.0,
                                    base=c * NC + co - j * NP,
                                    channel_multiplier=-1,
                                )
                            outp.append(p_sb)
                        return co, outp

                    def _emit_av(j, co, pexp_j):
                        for hh in range(2):
                            h = 2 * pp + hh             # head within quad
                            nc.tensor.matmul(
                                out=av[hh][0:DH + 1, co:NC],
                                lhsT=v1_sb[:, h, j, :],
                                rhs=pexp_j[hh][:, co:NC],
                                start=(j == 0), stop=(j == jmax - 1),
                                skip_group_check=True,
                            )

                    prev = None
                    for j in range(jmax):
                        cur = (j,) + _emit_st(j)
                        if prev is not None:
                            _emit_av(*prev)
                        prev = cur
                    _emit_av(*prev)

                    for hh in range(2):
                        avsb = avpool.tile([DH, NC], f32, name="avsb")
                        nc.vector.tensor_copy(out=avsb, in_=av[hh][0:DH, :])
                        den_sb = smalls.tile([1, NC], f32r, tag="den", name="den_sb")
                        nc.vector.tensor_copy(out=den_sb, in_=av[hh][DH:DH + 1, :])
                        norm_pending.append((avsb, den_sb, pair, hh, c))

        flush_normalizes()                              # last quad's items
        wo_sb = wo_sb_holder[0]

        # --- Wo projection: final[q, d] = sum_pair OT.T @ Wo + bo ------------
        for qi in range(NT):
            f_sb = fpool.tile([NP, D], f32, name="f_sb")
            for dc in range(NCH):
                wo_ps = ps_av.tile([NP, NC], f32, tag="av_ps", name="wo_ps")
                for pp in range(NPAIR):
                    nc.tensor.matmul(
                        out=wo_ps,
                        lhsT=ot_sb[:, pp, qi * NP:(qi + 1) * NP],
                        rhs=wo_sb[:, pp, dc * NC:(dc + 1) * NC],
                        start=(pp == 0), stop=(pp == NPAIR - 1),
                    )
                nc.vector.tensor_add(
                    out=f_sb[:, dc * NC:(dc + 1) * NC],
                    in0=wo_ps,
                    in1=bo_bc[:, dc * NC:(dc + 1) * NC],
                )
            nc.sync.dma_start(out=out[qi * NP:(qi + 1) * NP, :], in_=f_sb)


def _split_waits(nc, max_waits=1):
    """Walrus on this target allows one sync-wait per instruction; hoist
    extras onto no-ops inserted just before the offending instruction."""
    for f in nc.m.functions:
        for b in f.blocks:
            insts = b.instructions
            new = []
            changed = False
            for inst in insts:
                si = inst.sync_info
                if si is not None and len(si.on_wait) > max_waits:
                    waits = list(si.on_wait)
                    extra, keep = waits[:-max_waits], waits[-max_waits:]
                    for j, w in enumerate(extra):
                        new.append(mybir.InstNoOp(
                            name=f"{inst.name}-wnop{j}",
                            sync_info=mybir.SyncInfo(on_wait=[w], on_update=[]),
                            engine=inst.engine,
                            bass_nofuse=True,
                        ))
                    inst.sync_info = mybir.SyncInfo(
                        on_wait=keep, on_update=list(si.on_update))
                    changed = True
                new.append(inst)
            if changed:
                b.instructions = new


def make_in_maps(x, Wq, Wk, Wv, Wo, bo):
    scale = np.float32(DH) ** np.float32(-0.5)
    # [H, D, DH] -> [D, H*DH]; fold the 1/sqrt(DH) score scale into Wq
    wq_m = np.ascontiguousarray(
        Wq.transpose(1, 0, 2).reshape(D, H * DH) * scale).astype(np.float32)
    wk_m = np.ascontiguousarray(Wk.transpose(1, 0, 2).reshape(D, H * DH)).astype(np.float32)
    wv_m = np.ascontiguousarray(Wv.transpose(1, 0, 2).reshape(D, H * DH)).astype(np.float32)
    wo_m = np.ascontiguousarray(Wo).astype(np.float32)
    bo_m = np.ascontiguousarray(bo.reshape(1, D)).astype(np.float32)
    return [
        {
            "xt": np.ascontiguousarray(np.asarray(x[b]).T).astype(np.float32),
            "wq": wq_m, "wk": wk_m, "wv": wv_m, "wo": wo_m, "bo": bo_m,
        }
        for b in range(B)
    ]


_NC_CACHE = []


def kernel(x, Wq, Wk, Wv, Wo, bo):
    from concourse.bass_utils import run_bass_kernel_spmd

    x = np.asarray(x)
    if not _NC_CACHE:
        _NC_CACHE.append(build_nc())
    nc = _NC_CACHE[0]
    in_maps = make_in_maps(x, np.asarray(Wq), np.asarray(Wk), np.asarray(Wv),
                           np.asarray(Wo), np.asarray(bo))
    res = run_bass_kernel_spmd(nc, in_maps, core_ids=list(range(B)))
    return np.stack([res.results[b]["out"] for b in range(B)]).astype(np.float32)



# revision 4
# speedup vs baseline: 1.1007x; 1.1007x over previous
"""Multi-head causal self-attention on 8 Trainium2 NeuronCores.

Problem: B=8, T=1024, D=1024, H=16 heads, DH=64.
    q,k,v = einsum('btd,hdk->bhtk', x, W{q,k,v})
    scores = q @ k.T / sqrt(DH), causal mask, softmax
    out = (softmax @ v) reshaped -> [B,T,H*DH] @ Wo + bo

Sharding: batch-parallel, one batch element per core (B == n_cores == 8).
No collectives; weights replicated to every core.

v2 (bf16): all matmul operands are bfloat16 (rel err ~4e-3 vs the 2e-2
gate).  On this hardware a matmul instruction costs ~free_size cycles
regardless of dtype, but the implicit per-matmul LDWEIGHTS is ~4x cheaper
for 2-byte weights (~70ns vs ~285ns for a 128-row stationary), DMA bytes
halve, and fp32-mode power throttling (30% of the baseline ran at a 50%
util cap) is avoided.  walrus ignores InstMatmult.ldweights=False and
--enable-ldw-opt crashes codegen, so every matmul self-loads; the layout
below minimizes ldw rows instead.

Per-core dataflow:
  xt [d,t] host-transposed, d on partitions.
  V-pass (xt stationary): V[t, h*dh] for ALL heads in [128t, 1024] psum
    tiles, 2 x 512-free matmuls per (tt,kd) ldw -> v1[t, h, tt, dh+1]
    with a ones column (row dh of the AV psum then yields the softmax
    denominator for free).
  QK-pass (weight stationary): QT/KT come out directly as [128(2 heads
    pair-packed on dh), t] -- no transposes.
  Attention per pair, staggered one s-tile: ST[s,q] = KT_j.T @ QT with
    exact causal trim (q >= j*128 only), exp on ACT (no max-subtraction;
    scores are O(6)), diagonal-block mask via gpsimd affine_select, then
    AV accumulates (V|1).T @ exp(ST) into [65, 512] psums per (head,
    q-chunk).  QK(p+1) projections are emitted between attention pairs
    so the PE always has ~2x more queued work than ACT needs to keep up.
  Normalization is deferred: unnormalized AV + denominator rows park in
    SBUF; phase 3 runs batched ACT reciprocals (ONE table switch -- Exp
    and Reciprocal never share an ACT table), PE ones-matmul broadcasts,
    DVE column-scale, ordered c0-chunks-first so the Wo projection of
    q-tiles 0..3 overlaps the c1 normalizations.
  Wo: final[q,d] = sum_pp OT[:,pp,q].T @ Wo[pp-rows, d] + bo, f32 out.

This walrus build allows ONE sync-wait per instruction, so a
post-scheduling pass hoists extra waits onto inserted PE no-ops.
"""

import sys

for _p in ("/opt/trn_rl_repo", "/root/.axon_site/_ro/trn_rl_repo"):
    if _p not in sys.path:
        sys.path.insert(0, _p)

import numpy as np

import concourse.bass as bass
import concourse.mybir as mybir
import concourse.tile as tile

f32 = mybir.dt.float32
bf16 = mybir.dt.bfloat16

B, T, D, H, DH = 8, 1024, 1024, 16, 64
NP = 128            # partitions
NC = 512            # matmul free-dim chunk (moving-operand max)
KT_ = D // NP       # 8 contraction tiles over d
NT = T // NP        # 8 tiles over t (s and q tiles)
NCH = T // NC       # 2 free-dim chunks over q
NPAIR = H // 2      # 8 head pairs (QT/KT pack 2 heads on partitions)


def build_nc(split_waits=True):
    nc = bass.Bass(trn_type="TRN2")
    xt = nc.dram_tensor("xt", [D, T], bf16, kind="ExternalInput")
    wq = nc.dram_tensor("wq", [D, H * DH], bf16, kind="ExternalInput")
    wk = nc.dram_tensor("wk", [D, H * DH], bf16, kind="ExternalInput")
    wv = nc.dram_tensor("wv", [D, H * DH], bf16, kind="ExternalInput")
    wo = nc.dram_tensor("wo", [H * DH, D], bf16, kind="ExternalInput")
    bo = nc.dram_tensor("bo", [1, D], f32, kind="ExternalInput")
    out = nc.dram_tensor("out", [T, D], f32, kind="ExternalOutput")

    with tile.TileContext(nc) as tc:
        _mha(tc, nc, xt, wq, wk, wv, wo, bo, out)

    if split_waits:
        _split_waits(nc)
    return nc


def _mha(tc, nc, xt, wq, wk, wv, wo, bo, out):
    import contextlib

    ctx = contextlib.ExitStack()
    singles = ctx.enter_context(tc.tile_pool(name="singles", bufs=1))
    bigpool = ctx.enter_context(tc.tile_pool(name="bigpool", bufs=1))
    wpool = ctx.enter_context(tc.tile_pool(name="wpool", bufs=1))
    pexpool = ctx.enter_context(tc.tile_pool(name="pexpool", bufs=2))
    recpool = ctx.enter_context(tc.tile_pool(name="recpool", bufs=4))
    bcpool = ctx.enter_context(tc.tile_pool(name="bcpool", bufs=2))
    fpool = ctx.enter_context(tc.tile_pool(name="fpool", bufs=2))

    def act_recip(out_ap, in_ap):
        """ACT-engine reciprocal via raw InstActivation (nc.scalar.activation
        refuses Reciprocal; ~1.5e-6 rel err on our denominator range)."""
        ins = [nc.scalar.lower_ap(in_ap)]
        for arg in (0.0, 1.0, 0.0):                     # bias, scale, alpha
            ins.append(mybir.ImmediateValue(dtype=f32, value=arg))
        nc.scalar.add_instruction(mybir.InstActivation(
            name=nc.get_next_instruction_name(),
            func=mybir.ActivationFunctionType.Reciprocal,
            ins=ins,
            outs=[nc.scalar.lower_ap(out_ap)],
        ))

    with ctx:
        # --- resident tiles --------------------------------------------------
        onesf = singles.tile([NP, 1], f32)
        nc.vector.memset(onesf, 1.0)
        ones_row = singles.tile([1, DH], bf16)           # K=1 bcast matmul lhsT
        nc.vector.tensor_copy(out=ones_row, in_=onesf[0:1, 0:1].to_broadcast((1, DH)))
        bo_bc = singles.tile([NP, D], f32)
        nc.sync.dma_start(out=bo_bc, in_=bo[0:1, :].to_broadcast((NP, D)))

        # x^T: [p, kd, t]; chunked DMA so the first matmuls start early.
        # Shares a 2MB slot with Wo (only needed after QK consumed x^T).
        xt_sb = bigpool.tile([NP, KT_, T], bf16, tag="big", name="xt_sb")
        for kd in range(KT_):
            nc.sync.dma_start(out=xt_sb[:, kd, :], in_=xt[kd * NP:(kd + 1) * NP, :])
        wq_sb = wpool.tile([NP, KT_, H * DH], bf16, tag="wq")
        wk_sb = wpool.tile([NP, KT_, H * DH], bf16, tag="wk")
        wv_sb = wpool.tile([NP, KT_, H * DH], bf16, tag="wv")
        nc.sync.dma_start(out=wv_sb, in_=wv.rearrange("(kt p) c -> p kt c", p=NP))
        nc.sync.dma_start(out=wq_sb, in_=wq.rearrange("(kt p) c -> p kt c", p=NP))
        nc.sync.dma_start(out=wk_sb, in_=wk.rearrange("(kt p) c -> p kt c", p=NP))

        qt_sb = singles.tile([NP, NPAIR, T], bf16, name="qt_sb")
        kt_sb = singles.tile([NP, NPAIR, T], bf16, name="kt_sb")
        v1_sb = singles.tile([NP, H, NT, DH + 1], bf16, name="v1_sb")
        ot_sb = singles.tile([NP, NPAIR, T], bf16, name="ot_sb")
        # unnormalized AV + denominator row: [65, h, c, 512]
        avsb = singles.tile([DH + 1, H, NCH, NC], bf16, name="avsb")

        nc.vector.tensor_copy(
            out=v1_sb[:, :, :, DH:DH + 1],
            in_=onesf.to_broadcast((NP, H, NT, 1)))

        # --- V-pass: V[t, h*dh] for all heads, xt stationary -----------------
        with tc.tile_pool(name="ps_v", bufs=2, space="PSUM") as ps_v:
            for tt in range(NT):
                psv = ps_v.tile([NP, H, DH], f32, tag="v", name="psv")
                for kd in range(KT_):
                    for half in range(2):
                        nc.tensor.matmul(
                            out=psv[:, half * 8:(half + 1) * 8, :],
                            lhsT=xt_sb[:, kd, tt * NP:(tt + 1) * NP],
                            rhs=wv_sb[:, kd, half * NC:(half + 1) * NC],
                            start=(kd == 0), stop=(kd == KT_ - 1),
                        )
                nc.vector.tensor_copy(out=v1_sb[:, :, tt, 0:DH], in_=psv)

        psctx = contextlib.ExitStack()
        ps_qk = psctx.enter_context(tc.tile_pool(name="ps_qk", bufs=2, space="PSUM"))
        ps_st = psctx.enter_context(tc.tile_pool(name="ps_st", bufs=2, space="PSUM"))
        ps_av = psctx.enter_context(tc.tile_pool(name="ps_av", bufs=1, space="PSUM"))

        def emit_qk(pair):
            for w_sb, dst in ((wq_sb, qt_sb), (wk_sb, kt_sb)):
                for c in range(NCH):
                    ps = ps_qk.tile([NP, NC], f32, tag="qk", name="qk_ps")
                    for kd in range(KT_):
                        nc.tensor.matmul(
                            out=ps,
                            lhsT=w_sb[:, kd, pair * NP:(pair + 1) * NP],
                            rhs=xt_sb[:, kd, c * NC:(c + 1) * NC],
                            start=(kd == 0), stop=(kd == KT_ - 1),
                        )
                    nc.vector.tensor_copy(
                        out=dst[:, pair, c * NC:(c + 1) * NC], in_=ps)

        av_tiles = {}

        def emit_att(pair):
            # av psums [65, 512] per (hh, c); reused ring=1 across pairs
            for hh in range(2):
                for c in range(NCH):
                    av_tiles[(hh, c)] = ps_av.tile(
                        [DH + 1, NC], f32, tag=f"av{hh}{c}", name="av_ps")

            def emit_st(j):
                a0 = j * NP
                outp = []
                for hh in range(2):
                    hp = hh * DH
                    px = pexpool.tile([NP, T], bf16, tag=f"px{hh}", name="px")
                    for c in range(NCH):
                        lo = max(a0, c * NC)
                        if lo >= (c + 1) * NC:
                            continue
                        st = ps_st.tile([NP, NC], f32, tag="st", name="st_ps")
                        nc.tensor.matmul(
                            out=st[:, lo - c * NC:NC],
                            lhsT=kt_sb[hp:hp + DH, pair, a0:a0 + NP],
                            rhs=qt_sb[hp:hp + DH, pair, lo:(c + 1) * NC],
                            start=True, stop=True,
                        )
                        nc.scalar.activation(
                            out=px[:, lo:(c + 1) * NC],
                            in_=st[:, lo - c * NC:NC],
                            func=mybir.ActivationFunctionType.Exp)
                    # causal mask on the diagonal 128-col block only
                    nc.gpsimd.affine_select(
                        out=px[:, a0:a0 + NP], in_=px[:, a0:a0 + NP],
                        pattern=[[1, NP]],
                        compare_op=mybir.AluOpType.is_ge,
                        fill=0.0, base=0, channel_multiplier=-1,
                    )
                    outp.append(px)
                return outp

            def emit_av(j, pexp_j):
                a0 = j * NP
                for hh in range(2):
                    h = 2 * pair + hh
                    for c in range(NCH):
                        lo = max(a0, c * NC)
                        if lo >= (c + 1) * NC:
                            continue
                        nc.tensor.matmul(
                            out=av_tiles[(hh, c)][:, lo - c * NC:NC],
                            lhsT=v1_sb[:, h, j, :],
                            rhs=pexp_j[hh][:, lo:(c + 1) * NC],
                            start=(j == 0),
                            stop=(j == (NT - 1 if c else NT // NCH - 1)),
                            skip_group_check=True,
                        )

            prev = None
            for j in range(NT):
                cur = (j, emit_st(j))
                if prev is not None:
                    emit_av(*prev)
                prev = cur
            emit_av(*prev)

            for hh in range(2):
                h = 2 * pair + hh
                for c in range(NCH):
                    nc.vector.tensor_copy(
                        out=avsb[:, h, c, :], in_=av_tiles[(hh, c)])

        # QK interleaved one pair ahead of attention
        emit_qk(0)
        for pair in range(NPAIR):
            if pair + 1 < NPAIR:
                emit_qk(pair + 1)
            emit_att(pair)

        # Wo DMA into x^T's slot (x^T fully consumed by the QK pass)
        wo_sb = bigpool.tile([NP, NPAIR, D], bf16, tag="big", name="wo_sb")
        nc.sync.dma_start(out=wo_sb, in_=wo.rearrange("(kt p) d -> p kt d", p=NP))

        psctx.close()  # release ps_qk/ps_st/ps_av banks
        with tc.tile_pool(name="ps_wo", bufs=2, space="PSUM") as ps_wo, \
             tc.tile_pool(name="ps_bc", bufs=2, space="PSUM") as ps_bc:

            # --- deferred softmax normalization, c0 chunks first -------------
            def emit_norm(c):
                recips = []
                for h in range(H):
                    r = recpool.tile([1, NC], bf16, tag="rec", name="recip")
                    act_recip(r, avsb[DH:DH + 1, h, c, :])
                    recips.append(r)
                for h, r in enumerate(recips):
                    bc_ps = ps_bc.tile([DH, NC], f32, tag="bc", name="bc_ps")
                    nc.tensor.matmul(
                        out=bc_ps, lhsT=ones_row, rhs=r, start=True, stop=True)
                    bc = bcpool.tile([DH, NC], f32, tag="bc", name="bc_sb")
                    nc.vector.tensor_copy(out=bc, in_=bc_ps)
                    nc.vector.tensor_mul(
                        out=ot_sb[(h % 2) * DH:(h % 2 + 1) * DH,
                                  h // 2, c * NC:(c + 1) * NC],
                        in0=avsb[0:DH, h, c, :],
                        in1=bc,
                    )

            def emit_wo(qi):
                f_sb = fpool.tile([NP, D], f32, name="f_sb")
                for dc in range(NCH):
                    ps = ps_wo.tile([NP, NC], f32, tag=f"wo{dc}", name="wo_ps")
                    for pp in range(NPAIR):
                        nc.tensor.matmul(
                            out=ps,
                            lhsT=ot_sb[:, pp, qi * NP:(qi + 1) * NP],
                            rhs=wo_sb[:, pp, dc * NC:(dc + 1) * NC],
                            start=(pp == 0), stop=(pp == NPAIR - 1),
                        )
                    nc.vector.tensor_add(
                        out=f_sb[:, dc * NC:(dc + 1) * NC],
                        in0=ps,
                        in1=bo_bc[:, dc * NC:(dc + 1) * NC],
                    )
                nc.sync.dma_start(out=out[qi * NP:(qi + 1) * NP, :], in_=f_sb)

            emit_norm(0)
            for qi in range(NT // 2):
                emit_wo(qi)
            emit_norm(1)
            for qi in range(NT // 2, NT):
                emit_wo(qi)


def _split_waits(nc, max_waits=1):
    """Walrus on this target allows one sync-wait per instruction; hoist
    extras onto no-ops inserted just before the offending instruction."""
    for f in nc.m.functions:
        for b in f.blocks:
            insts = b.instructions
            new = []
            changed = False
            for inst in insts:
                si = inst.sync_info
                if si is not None and len(si.on_wait) > max_waits:
                    waits = list(si.on_wait)
                    extra, keep = waits[:-max_waits], waits[-max_waits:]
                    for j, w in enumerate(extra):
                        new.append(mybir.InstNoOp(
                            name=f"{inst.name}-wnop{j}",
                            sync_info=mybir.SyncInfo(on_wait=[w], on_update=[]),
                            engine=inst.engine,
                            bass_nofuse=True,
                        ))
                    inst.sync_info = mybir.SyncInfo(
                        on_wait=keep, on_update=list(si.on_update))
                    changed = True
                new.append(inst)
            if changed:
                b.instructions = new


def make_in_maps(x, Wq, Wk, Wv, Wo, bo):
    import ml_dtypes
    nbf = ml_dtypes.bfloat16
    scale = np.float32(DH) ** np.float32(-0.5)
    # [H, D, DH] -> [D, H*DH]; fold the 1/sqrt(DH) score scale into Wq
    wq_m = np.ascontiguousarray(
        np.asarray(Wq).transpose(1, 0, 2).reshape(D, H * DH) * scale).astype(nbf)
    wk_m = np.ascontiguousarray(
        np.asarray(Wk).transpose(1, 0, 2).reshape(D, H * DH)).astype(nbf)
    wv_m = np.ascontiguousarray(
        np.asarray(Wv).transpose(1, 0, 2).reshape(D, H * DH)).astype(nbf)
    wo_m = np.ascontiguousarray(np.asarray(Wo)).astype(nbf)
    bo_m = np.ascontiguousarray(np.asarray(bo).reshape(1, D)).astype(np.float32)
    return [
        {
            "xt": np.ascontiguousarray(np.asarray(x[b]).T).astype(nbf),
            "wq": wq_m, "wk": wk_m, "wv": wv_m, "wo": wo_m, "bo": bo_m,
        }
        for b in range(B)
    ]


_NC_CACHE = []


def kernel(x, Wq, Wk, Wv, Wo, bo):
    from concourse.bass_utils import run_bass_kernel_spmd

    x = np.asarray(x)
    if not _NC_CACHE:
        _NC_CACHE.append(build_nc())
    nc = _NC_CACHE[0]
    in_maps = make_in_maps(x, np.asarray(Wq), np.asarray(Wk), np.asarray(Wv),
                           np.asarray(Wo), np.asarray(bo))
    res = run_bass_kernel_spmd(nc, in_maps, core_ids=list(range(B)))
    return np.stack([res.results[b]["out"] for b in range(B)]).astype(np.float32)


# revision 15
# speedup vs baseline: 1.1364x; 1.0324x over previous
"""Multi-head causal self-attention on 8 Trainium2 NeuronCores.

Problem: B=8, T=1024, D=1024, H=16 heads, DH=64.
    q,k,v = einsum('btd,hdk->bhtk', x, W{q,k,v})
    scores = q @ k.T / sqrt(DH), causal mask, softmax
    out = (softmax @ v) reshaped -> [B,T,H*DH] @ Wo + bo

Sharding: batch-parallel, one batch element per core (B == n_cores == 8).
No collectives; weights replicated to every core.

v2 (bf16): all matmul operands are bfloat16 (rel err ~4e-3 vs the 2e-2
gate).  On this hardware a matmul instruction costs ~free_size cycles
regardless of dtype, but the implicit per-matmul LDWEIGHTS is ~4x cheaper
for 2-byte weights (~70ns vs ~285ns for a 128-row stationary), DMA bytes
halve, and fp32-mode power throttling (30% of the baseline ran at a 50%
util cap) is avoided.  walrus ignores InstMatmult.ldweights=False and
--enable-ldw-opt crashes codegen, so every matmul self-loads; the layout
below minimizes ldw rows instead.

Per-core dataflow:
  xt [d,t] host-transposed, d on partitions.
  V-pass (xt stationary): V[t, h*dh] for ALL heads in [128t, 1024] psum
    tiles, 2 x 512-free matmuls per (tt,kd) ldw -> v1[t, h, tt, dh+1]
    with a ones column (row dh of the AV psum then yields the softmax
    denominator for free).
  QK-pass (weight stationary): QT/KT come out directly as [128(2 heads
    pair-packed on dh), t] -- no transposes.
  Attention per pair, staggered one s-tile: ST[s,q] = KT_j.T @ QT with
    exact causal trim (q >= j*128 only), exp on ACT (no max-subtraction;
    scores are O(6)), diagonal-block mask via gpsimd affine_select, then
    AV accumulates (V|1).T @ exp(ST) into [65, 512] psums per (head,
    q-chunk).  QK(p+1) projections are emitted between attention pairs
    so the PE always has ~2x more queued work than ACT needs to keep up.
  Normalization is deferred: unnormalized AV + denominator rows park in
    SBUF; phase 3 runs batched ACT reciprocals (ONE table switch -- Exp
    and Reciprocal never share an ACT table), PE ones-matmul broadcasts,
    DVE column-scale, ordered c0-chunks-first so the Wo projection of
    q-tiles 0..3 overlaps the c1 normalizations.
  Wo: final[q,d] = sum_pp OT[:,pp,q].T @ Wo[pp-rows, d] + bo, f32 out.

This walrus build allows ONE sync-wait per instruction, so a
post-scheduling pass hoists extra waits onto inserted PE no-ops.
"""

import sys

for _p in ("/opt/trn_rl_repo", "/root/.axon_site/_ro/trn_rl_repo"):
    if _p not in sys.path:
        sys.path.insert(0, _p)

import numpy as np

import concourse.bass as bass
import concourse.mybir as mybir
import concourse.tile as tile

f32 = mybir.dt.float32
bf16 = mybir.dt.bfloat16

B, T, D, H, DH = 8, 1024, 1024, 16, 64
NP = 128            # partitions
NC = 512            # matmul free-dim chunk (moving-operand max)
KT_ = D // NP       # 8 contraction tiles over d
NT = T // NP        # 8 tiles over t (s and q tiles)
NCH = T // NC       # 2 free-dim chunks over q
NPAIR = H // 2      # 8 head pairs (QT/KT pack 2 heads on partitions)


def build_nc(split_waits=True):
    nc = bass.Bass(trn_type="TRN2")
    xt = nc.dram_tensor("xt", [D, T], bf16, kind="ExternalInput")
    wq = nc.dram_tensor("wq", [D, H * DH], bf16, kind="ExternalInput")
    wk = nc.dram_tensor("wk", [D, H * DH], bf16, kind="ExternalInput")
    wv = nc.dram_tensor("wv", [D, H * DH], bf16, kind="ExternalInput")
    wo = nc.dram_tensor("wo", [H * DH, D], bf16, kind="ExternalInput")
    bo = nc.dram_tensor("bo", [1, D], f32, kind="ExternalInput")
    out = nc.dram_tensor("out", [T, D], f32, kind="ExternalOutput")

    with tile.TileContext(nc) as tc:
        _mha(tc, nc, xt, wq, wk, wv, wo, bo, out)

    if split_waits:
        _split_waits(nc)
    return nc


def _mha(tc, nc, xt, wq, wk, wv, wo, bo, out):
    import contextlib

    ctx = contextlib.ExitStack()
    singles = ctx.enter_context(tc.tile_pool(name="singles", bufs=1))
    bigpool = ctx.enter_context(tc.tile_pool(name="bigpool", bufs=1))
    wpool = ctx.enter_context(tc.tile_pool(name="wpool", bufs=1))
    pexpool = ctx.enter_context(tc.tile_pool(name="pexpool", bufs=2))
    bcpool = ctx.enter_context(tc.tile_pool(name="bcpool", bufs=2))
    fpool = ctx.enter_context(tc.tile_pool(name="fpool", bufs=2))

    def act_recip(out_ap, in_ap):
        """ACT-engine reciprocal via raw InstActivation (nc.scalar.activation
        refuses Reciprocal; ~1.5e-6 rel err on our denominator range)."""
        ins = [nc.scalar.lower_ap(in_ap)]
        for arg in (0.0, 1.0, 0.0):                     # bias, scale, alpha
            ins.append(mybir.ImmediateValue(dtype=f32, value=arg))
        nc.scalar.add_instruction(mybir.InstActivation(
            name=nc.get_next_instruction_name(),
            func=mybir.ActivationFunctionType.Reciprocal,
            ins=ins,
            outs=[nc.scalar.lower_ap(out_ap)],
        ))

    with ctx:
        # --- resident tiles --------------------------------------------------
        onesf = singles.tile([NP, 1], f32)
        nc.vector.memset(onesf, 1.0)
        # K=1 bcast matmul lhsT; row 64 sliced so base partition matches the
        # denominator row (partition 64 of avsb)
        ones2d = singles.tile([NP, DH], bf16)
        nc.vector.tensor_copy(out=ones2d, in_=onesf.to_broadcast((NP, DH)))
        ones_row = ones2d[DH:DH + 1, :]
        bo_bc = singles.tile([NP, D], f32)
        nc.sync.dma_start(out=bo_bc, in_=bo[0:1, :].to_broadcast((NP, D)))

        # x^T: [p, kd, t]; all inputs DMA'd in per-kd chunks, interleaved in
        # first-use order so the first QK matmuls start within a few us.
        # x^T shares a 2MB slot with Wo (only needed after QK consumed x^T).
        xt_sb = bigpool.tile([NP, KT_, T], bf16, tag="big", name="xt_sb")
        wq_sb = wpool.tile([NP, KT_, H * DH], bf16, tag="wq")
        wk_sb = wpool.tile([NP, KT_, H * DH], bf16, tag="wk")
        wv_sb = wpool.tile([NP, KT_, H * DH], bf16, tag="wv")
        for kd in range(KT_):
            nc.sync.dma_start(out=xt_sb[:, kd, :], in_=xt[kd * NP:(kd + 1) * NP, :])
            nc.sync.dma_start(out=wq_sb[:, kd, :], in_=wq[kd * NP:(kd + 1) * NP, :])
        for kd in range(KT_):
            nc.sync.dma_start(out=wk_sb[:, kd, :], in_=wk[kd * NP:(kd + 1) * NP, :])
        for kd in range(KT_):
            nc.sync.dma_start(out=wv_sb[:, kd, :], in_=wv[kd * NP:(kd + 1) * NP, :])

        qt_sb = singles.tile([NP, NPAIR, T], bf16, name="qt_sb")
        kt_sb = singles.tile([NP, NPAIR, T], bf16, name="kt_sb")
        v1_sb = singles.tile([NP, H, NT, DH + 1], bf16, name="v1_sb")
        ot_sb = singles.tile([NP, NPAIR, T], bf16, name="ot_sb")
        # unnormalized AV + denominator row: [65, h, c, 512]
        avsb = singles.tile([DH + 1, H, NCH, NC], bf16, name="avsb")

        nc.vector.tensor_copy(
            out=v1_sb[:, :, :, DH:DH + 1],
            in_=onesf.to_broadcast((NP, H, NT, 1)))

        psctx = contextlib.ExitStack()
        ps_qk = psctx.enter_context(tc.tile_pool(name="ps_qk", bufs=2, space="PSUM"))
        ps_st = psctx.enter_context(tc.tile_pool(name="ps_st", bufs=2, space="PSUM"))

        def emit_qk(pair):
            for w_sb, dst in ((wq_sb, qt_sb), (wk_sb, kt_sb)):
                for c in range(NCH):
                    ps = ps_qk.tile([NP, NC], f32, tag="qk", name="qk_ps")
                    for kd in range(KT_):
                        nc.tensor.matmul(
                            out=ps,
                            lhsT=w_sb[:, kd, pair * NP:(pair + 1) * NP],
                            rhs=xt_sb[:, kd, c * NC:(c + 1) * NC],
                            start=(kd == 0), stop=(kd == KT_ - 1),
                        )
                    nc.vector.tensor_copy(
                        out=dst[:, pair, c * NC:(c + 1) * NC], in_=ps)

        av_tiles = {}

        def emit_att(pair):
            # av psums [65, 512] per (hh, c); reused ring=1 across pairs
            for hh in range(2):
                for c in range(NCH):
                    av_tiles[(hh, c)] = ps_av.tile(
                        [DH + 1, NC], f32, tag=f"av{hh}{c}", name="av_ps")

            def emit_st(j):
                a0 = j * NP
                outp = []
                for hh in range(2):
                    hp = hh * DH
                    px = pexpool.tile([NP, T], bf16, tag=f"px{hh}", name="px")
                    for c in range(NCH):
                        lo = max(a0, c * NC)
                        if lo >= (c + 1) * NC:
                            continue
                        st = ps_st.tile([NP, NC], f32, tag="st", name="st_ps")
                        nc.tensor.matmul(
                            out=st[:, lo - c * NC:NC],
                            lhsT=kt_sb[hp:hp + DH, pair, a0:a0 + NP],
                            rhs=qt_sb[hp:hp + DH, pair, lo:(c + 1) * NC],
                            start=True, stop=True,
                        )
                        nc.scalar.activation(
                            out=px[:, lo:(c + 1) * NC],
                            in_=st[:, lo - c * NC:NC],
                            func=mybir.ActivationFunctionType.Exp)
                    # causal mask on the diagonal 128-col block only
                    nc.gpsimd.affine_select(
                        out=px[:, a0:a0 + NP], in_=px[:, a0:a0 + NP],
                        pattern=[[1, NP]],
                        compare_op=mybir.AluOpType.is_ge,
                        fill=0.0, base=0, channel_multiplier=-1,
                    )
                    outp.append(px)
                return outp

            def emit_av(j, pexp_j):
                a0 = j * NP
                for hh in range(2):
                    h = 2 * pair + hh
                    for c in range(NCH):
                        lo = max(a0, c * NC)
                        if lo >= (c + 1) * NC:
                            continue
                        nc.tensor.matmul(
                            out=av_tiles[(hh, c)][:, lo - c * NC:NC],
                            lhsT=v1_sb[:, h, j, :],
                            rhs=pexp_j[hh][:, lo:(c + 1) * NC],
                            start=(j == 0),
                            stop=(j == (NT - 1 if c else NT // NCH - 1)),
                            skip_group_check=True,
                        )

            prev = None
            for j in range(NT):
                cur = (j, emit_st(j))
                if prev is not None:
                    emit_av(*prev)
                prev = cur
            emit_av(*prev)

            for hh in range(2):
                h = 2 * pair + hh
                for c in range(NCH):
                    nc.vector.tensor_copy(
                        out=avsb[:, h, c, :], in_=av_tiles[(hh, c)])

        # QK for the first two pairs, then the V-pass (xt stationary, all
        # heads), then attention with QK kept two pairs ahead
        emit_qk(0)
        emit_qk(1)
        with tc.tile_pool(name="ps_v", bufs=2, space="PSUM") as ps_v:
            for tt in range(NT):
                psv = ps_v.tile([NP, H, DH], f32, tag="v", name="psv")
                for kd in range(KT_):
                    for half in range(2):
                        nc.tensor.matmul(
                            out=psv[:, half * 8:(half + 1) * 8, :],
                            lhsT=xt_sb[:, kd, tt * NP:(tt + 1) * NP],
                            rhs=wv_sb[:, kd, half * NC:(half + 1) * NC],
                            start=(kd == 0), stop=(kd == KT_ - 1),
                        )
                nc.vector.tensor_copy(out=v1_sb[:, :, tt, 0:DH], in_=psv)
        ps_av = psctx.enter_context(tc.tile_pool(name="ps_av", bufs=1, space="PSUM"))
        for pair in range(NPAIR):
            if pair + 2 < NPAIR:
                emit_qk(pair + 2)
            emit_att(pair)

        # Wo DMA into x^T's slot (x^T fully consumed by the QK pass)
        wo_sb = bigpool.tile([NP, NPAIR, D], bf16, tag="big", name="wo_sb")
        nc.sync.dma_start(out=wo_sb, in_=wo.rearrange("(kt p) d -> p kt d", p=NP))

        psctx.close()  # release ps_qk/ps_st/ps_av banks
        with tc.tile_pool(name="ps_wo", bufs=2, space="PSUM") as ps_wo, \
             tc.tile_pool(name="ps_bc", bufs=2, space="PSUM") as ps_bc:

            # --- deferred softmax normalization ------------------------------
            # Batched IN-PLACE reciprocals over the avsb denominator rows:
            # each instr covers all 16 heads, so its deps include pair 7 and
            # the tile scheduler cannot hoist it into the attention stream
            # (hoisted recips thrash the exp<->recip ACT tables mid-pipeline).
            for c in range(NCH):
                act_recip(avsb[DH:DH + 1, :, c, :], avsb[DH:DH + 1, :, c, :])

            def emit_norm(c):
                for h in range(H):
                    # PE-broadcast the recip'd denominator row to 64 rows
                    bc_ps = ps_bc.tile([DH, NC], f32, tag="bc", name="bc_ps")
                    nc.tensor.matmul(
                        out=bc_ps, lhsT=ones_row, rhs=avsb[DH:DH + 1, h, c, :],
                        start=True, stop=True)
                    nc.vector.tensor_mul(
                        out=ot_sb[(h % 2) * DH:(h % 2 + 1) * DH,
                                  h // 2, c * NC:(c + 1) * NC],
                        in0=avsb[0:DH, h, c, :],
                        in1=bc_ps,
                    )

            def emit_wo(qi):
                f_sb = fpool.tile([NP, D], f32, name="f_sb")
                for dc in range(NCH):
                    ps = ps_wo.tile([NP, NC], f32, tag=f"wo{dc}", name="wo_ps")
                    for pp in range(NPAIR):
                        nc.tensor.matmul(
                            out=ps,
                            lhsT=ot_sb[:, pp, qi * NP:(qi + 1) * NP],
                            rhs=wo_sb[:, pp, dc * NC:(dc + 1) * NC],
                            start=(pp == 0), stop=(pp == NPAIR - 1),
                        )
                    nc.vector.tensor_add(
                        out=f_sb[:, dc * NC:(dc + 1) * NC],
                        in0=ps,
                        in1=bo_bc[:, dc * NC:(dc + 1) * NC],
                    )
                nc.sync.dma_start(out=out[qi * NP:(qi + 1) * NP, :], in_=f_sb)

            emit_norm(0)
            for qi in range(NT // 2):
                emit_wo(qi)
            emit_norm(1)
            for qi in range(NT // 2, NT):
                emit_wo(qi)


def _split_waits(nc, max_waits=1):
    """Walrus on this target allows one sync-wait per instruction; hoist
    extras onto no-ops inserted just before the offending instruction."""
    for f in nc.m.functions:
        for b in f.blocks:
            insts = b.instructions
            new = []
            changed = False
            for inst in insts:
                si = inst.sync_info
                if si is not None and len(si.on_wait) > max_waits:
                    waits = list(si.on_wait)
                    extra, keep = waits[:-max_waits], waits[-max_waits:]
                    for j, w in enumerate(extra):
                        new.append(mybir.InstNoOp(
                            name=f"{inst.name}-wnop{j}",
                            sync_info=mybir.SyncInfo(on_wait=[w], on_update=[]),
                            engine=inst.engine,
                            bass_nofuse=True,
                        ))
                    inst.sync_info = mybir.SyncInfo(
                        on_wait=keep, on_update=list(si.on_update))
                    changed = True
                new.append(inst)
            if changed:
                b.instructions = new


def make_in_maps(x, Wq, Wk, Wv, Wo, bo):
    import ml_dtypes
    nbf = ml_dtypes.bfloat16
    scale = np.float32(DH) ** np.float32(-0.5)
    # [H, D, DH] -> [D, H*DH]; fold the 1/sqrt(DH) score scale into Wq
    wq_m = np.ascontiguousarray(
        np.asarray(Wq).transpose(1, 0, 2).reshape(D, H * DH) * scale).astype(nbf)
    wk_m = np.ascontiguousarray(
        np.asarray(Wk).transpose(1, 0, 2).reshape(D, H * DH)).astype(nbf)
    wv_m = np.ascontiguousarray(
        np.asarray(Wv).transpose(1, 0, 2).reshape(D, H * DH)).astype(nbf)
    wo_m = np.ascontiguousarray(np.asarray(Wo)).astype(nbf)
    bo_m = np.ascontiguousarray(np.asarray(bo).reshape(1, D)).astype(np.float32)
    return [
        {
            "xt": np.ascontiguousarray(np.asarray(x[b]).T).astype(nbf),
            "wq": wq_m, "wk": wk_m, "wv": wv_m, "wo": wo_m, "bo": bo_m,
        }
        for b in range(B)
    ]


_NC_CACHE = []


def kernel(x, Wq, Wk, Wv, Wo, bo):
    from concourse.bass_utils import run_bass_kernel_spmd

    x = np.asarray(x)
    if not _NC_CACHE:
        _NC_CACHE.append(build_nc())
    nc = _NC_CACHE[0]
    in_maps = make_in_maps(x, np.asarray(Wq), np.asarray(Wk), np.asarray(Wv),
                           np.asarray(Wo), np.asarray(bo))
    res = run_bass_kernel_spmd(nc, in_maps, core_ids=list(range(B)))
    return np.stack([res.results[b]["out"] for b in range(B)]).astype(np.float32)
